# revision 30
# baseline (speedup 1.0000x reference)
"""Trainium2 Bass kernel for a GPT-style decoder block (S=2048, E=2048, H=16, D=128).

Strategy (8 NeuronCores, SPMD-uniform program):
- Sequence-parallel LN1/QKV: core c owns rows 256c..256c+255 and computes
  q,k,v (bf16) for all 16 heads of its rows.
- Two AllToAlls re-shard (q,k,v) from sequence-sharded to head-sharded:
  round m moves heads 8m..8m+7 (head 8m+c lands on core c).
- Head-sharded causal attention: each core runs full-sequence attention
  for its 2 heads, computing only the causal lower-triangle score blocks
  (identical work on every core, so the program stays uniform).
- Attention out-projection is computed as per-core partials (contraction
  over the 2 owned heads only) and summed with two pipelined bf16
  ReduceScatters (even row-blocks first, then odd), which also return
  the result to sequence sharding.
- LN2 + FFN run sequence-parallel; both 128-row chunks are processed per
  streamed weight tile so wfc/wpf are read from HBM exactly once.

Matmuls accumulate in fp32 PSUM; the residual stream stays fp32 in SBUF.
LayerNorm scales fold into the following weights host-side.
"""

import numpy as np
import ml_dtypes

import concourse.mybir as mybir
import concourse.tile as tile
from concourse import bacc
from concourse.bass_utils import run_bass_kernel_spmd

P = 128
S, E, H, D = 2048, 2048, 16, 128
FH = 4 * E
NCORES = 8
NBLK = 16            # S / P row blocks
EC = 16              # E / P contraction chunks
FC2 = 64             # FH / P
BF = mybir.dt.bfloat16
F32 = mybir.dt.float32
EPS = 1e-5
SCALE = 1.0 / np.sqrt(D)
AF = mybir.ActivationFunctionType
ALU = mybir.AluOpType

KQB = P * 256        # bf16 elements of one [128, 256] k/q block in a chunk
CH = 3 * KQB         # chunk elements: k + q + v (all bf16)


def _layer_norm(nc, small, x_sb, h_out, eps_tile, tag):
    """h_out[P, E] = (x - mean(x)) * rsqrt(var(x) + eps), rowwise over E."""
    stats = small.tile([P, 4, 6], F32, name=f"stats_{tag}", tag="stats")
    for g in range(4):
        nc.vector.bn_stats(out=stats[:, g, :], in_=x_sb[:, g * 512:(g + 1) * 512])
    mv = small.tile([P, 2], F32, name=f"mv_{tag}", tag="mv")
    nc.vector.bn_aggr(out=mv[:], in_=stats[:])
    std = small.tile([P, 1], F32, name=f"std_{tag}", tag="std")
    nc.scalar.activation(out=std[:], in_=mv[:, 1:2], func=AF.Sqrt,
                         bias=eps_tile[:], scale=1.0)
    rstd = small.tile([P, 1], F32, name=f"rstd_{tag}", tag="rstd")
    nc.vector.reciprocal(out=rstd[:], in_=std[:])
    nc.vector.tensor_scalar(
        out=h_out[:], in0=x_sb[:], scalar1=mv[:, 0:1], scalar2=rstd[:],
        op0=ALU.subtract, op1=ALU.mult)


def build_program():
    nc = bacc.Bacc()

    x_own = nc.dram_tensor("x_own", [2, P, E], F32, kind="ExternalInput")
    # wqk[m][i][e, ec, d]: i<8 -> k head 8m+i, i>=8 -> q head 8m+i-8
    wqk = nc.dram_tensor("wqk", [2, 16, P, EC, P], BF, kind="ExternalInput")
    # wv[m][ec][e, cc, f]: round-m v cols 1024m+256cc+f
    wv = nc.dram_tensor("wv", [2, EC, P, 4, 256], BF, kind="ExternalInput")
    # wpa[i, mm, n, f] = w_proj_attn[512n+f, 128*(8mm+c)+i]  (per-core)
    wpa = nc.dram_tensor("wpa", [P, 2, 4, 512], BF, kind="ExternalInput")
    # wfc[f2][e, ec, f] = (w_fc*ln2)[128f2+f, 128ec+e]
    wfc = nc.dram_tensor("wfc", [FC2, P, EC, P], BF, kind="ExternalInput")
    # wpf[f2][i, n, f] = w_proj_ffn[512n+f, 128f2+i]
    wpf = nc.dram_tensor("wpf", [FC2, P, 4, 512], BF, kind="ExternalInput")
    # mask[t, dtb, qf] = 1 if 128*dtb + t <= qf else 0 (rank-independent)
    mask_in = nc.dram_tensor("mask", [P, 4, 512], BF, kind="ExternalInput")
    ident_in = nc.dram_tensor("ident", [P, P], BF, kind="ExternalInput")
    out_own = nc.dram_tensor("out_own", [2, P, E], F32, kind="ExternalOutput")

    with tile.TileContext(nc) as tc:
        _body(nc, tc, x_own, wqk, wv, wpa, wfc, wpf, mask_in, ident_in, out_own)
    nc.finalize()
    return nc


def _body(nc, tc, x_own, wqk, wv, wpa, wfc, wpf, mask_in, ident_in, out_own):
    with tc.tile_pool(name="resident", bufs=1) as res, \
         tc.tile_pool(name="small", bufs=4) as small, \
         tc.tile_pool(name="dram", bufs=1, space="DRAM") as dram:

        eps_tile = small.tile([P, 1], F32, name="eps_tile", tag="eps")
        nc.vector.memset(eps_tile[:], EPS)
        x_sb = []
        for j in range(2):
            x = res.tile([P, E], F32, name=f"x_sb{j}")
            nc.sync.dma_start(x[:], x_own[j])
            x_sb.append(x)
        ident = res.tile([P, P], BF, name="ident_sb")
        nc.sync.dma_start(ident[:], ident_in[:])
        mask_sb = res.tile([P, 4, 512], BF, name="mask_sb")

        hT = res.tile([P, EC, 256], BF, name="hT_sb")
        attnT = res.tile([P, 2, S], BF, name="attnT_sb")

        a2a_in = [dram.tile([NCORES, CH], BF, name=f"a2a_in{m}")
                  for m in range(2)]
        a2a_out = [dram.tile([NCORES, CH], BF, name=f"a2a_out{m}")
                   for m in range(2)]
        rs_in = [dram.tile([NCORES, P, E], BF, name=f"rs_in{i}") for i in range(2)]
        rs_out = [dram.tile([P, E], BF, name=f"rs_out{i}") for i in range(2)]

        # ---------- LN1 -> hT (bf16) ----------
        with tc.tile_pool(name="hbf", bufs=2) as hbf_pool, \
             tc.tile_pool(name="tps1", bufs=4, space="PSUM") as tps1:
            h_bfs = []
            for j in range(2):
                h_bf = hbf_pool.tile([P, E], BF, name="h_bf", tag="h_bf")
                _layer_norm(nc, small, x_sb[j], h_bf, eps_tile, f"ln1_{j}")
                h_bfs.append(h_bf)
            for e in range(EC):
                for j in range(2):
                    tp = tps1.tile([P, P], BF, name="tp1", tag="tp1")
                    nc.tensor.transpose(tp[:], h_bfs[j][:, e * P:(e + 1) * P],
                                        ident[:])
                    nc.vector.tensor_copy(hT[:, e, j * P:(j + 1) * P], tp[:])

        with tc.tile_wait_until(0.095):
            nc.scalar.dma_start(mask_sb[:], mask_in[:])

        # ---------- QKV rounds + AllToAll ----------
        # Per-round weights land in SBUF via 3 big ordered DMAs on the sync
        # queue (the in-order SEQ serializes them ahead of payload writes),
        # so the PE runs unthrottled once data arrives.
        with tc.tile_pool(name="wv_sb", bufs=2) as wv_pool, \
             tc.tile_pool(name="wqk_sb", bufs=2) as wqk_pool:
            wv_t = []
            wqk_t = []
            for m in range(2):
                from contextlib import nullcontext
                gate = tc.tile_wait_until(0.046) if m == 1 else nullcontext()
                with gate:
                    halves = []
                    for hh in range(2):
                        wt = wqk_pool.tile([P, 8, EC, P], BF,
                                           name=f"wqk_t{m}{hh}", tag="wqk_t")
                        nc.sync.dma_start(
                            wt[:], wqk[m, 8 * hh:8 * hh + 8].rearrange(
                                "c e f g -> e c f g"))
                        halves.append(wt)
                    wvt = wv_pool.tile([P, EC, 4, 256], BF, name=f"wv_t{m}",
                                       tag="wv_t")
                    nc.sync.dma_start(wvt[:],
                                      wv[m].rearrange("c e f g -> e c f g"))
                wv_t.append(wvt)
                wqk_t.append(halves)

                with tc.tile_pool(name=f"kvtmp{m}", bufs=4) as kv_pool, \
                     tc.tile_pool(name=f"qkps{m}", bufs=3, space="PSUM") as qkps, \
                     tc.tile_pool(name=f"vps{m}", bufs=1, space="PSUM") as vps:
                    for i in range(16):
                        wt = halves[i // 8]
                        ps = qkps.tile([P, 256], F32, name="qk_ps", tag="qk_ps")
                        for e in range(EC):
                            nc.tensor.matmul(ps[:], wt[:, i % 8, e, :],
                                             hT[:, e, :],
                                             start=(e == 0), stop=(e == EC - 1))
                        kq_sb = kv_pool.tile([P, P], BF, name="kq_sb",
                                             tag="kq_sb")
                        kq_sb2 = kv_pool.tile([P, P], BF, name="kq_sb2",
                                              tag="kq_sb2")
                        nc.scalar.copy(kq_sb[:], ps[:, :P])
                        nc.vector.tensor_copy(kq_sb2[:], ps[:, P:])
                        r, slot = i % 8, i // 8
                        dst = a2a_in[m][r, slot * KQB:(slot + 1) * KQB]
                        nc.sync.dma_start(
                            dst.rearrange("(d t) -> d t", d=P)[:, :P], kq_sb[:])
                        nc.sync.dma_start(
                            dst.rearrange("(d t) -> d t", d=P)[:, P:],
                            kq_sb2[:])
                    ps_v = [[vps.tile([P, 512], F32, name=f"v_ps{j}_{ccp}",
                                      tag=f"v_ps{j}_{ccp}") for ccp in range(2)]
                            for j in range(2)]
                    for e in range(EC):
                        for j in range(2):
                            for ccp in range(2):
                                nc.tensor.matmul(
                                    ps_v[j][ccp][:],
                                    hT[:, e, j * P:(j + 1) * P],
                                    wvt[:, e, 2 * ccp:2 * ccp + 2, :],
                                    start=(e == 0), stop=(e == EC - 1))
                    for j in range(2):
                        for ccp in range(2):
                            v_sb = kv_pool.tile([P, 512], BF, name="v_sb",
                                                tag="v_sb")
                            nc.vector.tensor_copy(v_sb[:], ps_v[j][ccp][:])
                            for half in range(4):
                                r = 4 * ccp + half
                                dst = a2a_in[m][r, 2 * KQB + j * (KQB // 2):
                                                2 * KQB + (j + 1) * (KQB // 2)]
                                nc.sync.dma_start(
                                    dst.rearrange("(t d) -> t d", t=P),
                                    v_sb[:, half * P:(half + 1) * P])
                nc.gpsimd.collective_compute(
                    "AllToAll", ALU.bypass,
                    replica_groups=[list(range(NCORES))],
                    ins=[a2a_in[m].opt()], outs=[a2a_out[m].opt()])

        # ---------- attention: 2 heads, causal-only blocks ----------
        with tc.tile_pool(name="kqv_in", bufs=2) as kqv_pool, \
             tc.tile_pool(name="expt", bufs=2) as exp_pool, \
             tc.tile_pool(name="attn_small", bufs=4) as asmall, \
             tc.tile_pool(name="scps", bufs=3, space="PSUM") as scps, \
             tc.tile_pool(name="atps", bufs=2, space="PSUM") as atps, \
             tc.tile_pool(name="trps", bufs=2, space="PSUM") as trps:
            for m in range(2):
                kT = kqv_pool.tile([P, NBLK, P], BF, name="kT", tag="kT")
                nc.sync.dma_start(
                    kT[:],
                    a2a_out[m][:, :KQB].rearrange("r (d t) -> d r t", d=P))
                qT = kqv_pool.tile([P, S], BF, name="qT", tag="qT")
                nc.sync.dma_start(
                    qT[:],
                    a2a_out[m][:, KQB:2 * KQB].rearrange(
                        "r (d t) -> d r t", d=P))
                v_sb = kqv_pool.tile([P, 2, NCORES, D + 2], BF,
                                     name="v_all", tag="v_all")
                nc.vector.memset(v_sb[:, :, :, D:D + 1], 1.0)
                for j in range(2):
                    nc.scalar.dma_start(
                        v_sb[:, j, :, :D],
                        a2a_out[m][:, 2 * KQB + j * (KQB // 2):
                                   2 * KQB + (j + 1) * (KQB // 2)].rearrange(
                            "r (t d) -> t r d", t=P))

                expT = [None, None]  # pipeline: scores(g) || attnV(g-1)
                for g in range(5):
                    if g < 4:
                        ex = exp_pool.tile([P, NBLK, 512], BF, name="expT",
                                           tag="expT")
                        expT[g % 2] = ex
                        for tb in range(4 * g + 4):
                            dtb = tb - 4 * g
                            q0 = max(dtb, 0) * P  # causal: skip left of diag
                            ps = scps.tile([P, 512], F32, name="sc_ps",
                                           tag="sc_ps")
                            nc.tensor.matmul(
                                ps[:, q0:], kT[:, tb, :],
                                qT[:, g * 512 + q0:(g + 1) * 512],
                                start=True, stop=True)
                            nc.scalar.activation(
                                out=ex[:, tb, q0:], in_=ps[:, q0:],
                                func=AF.Exp, scale=float(SCALE))
                            if dtb >= 0:
                                nc.vector.tensor_mul(
                                    ex[:, tb, q0:], ex[:, tb, q0:],
                                    mask_sb[:, dtb, q0:])
                    if g > 0:
                        gg = g - 1
                        ex = expT[gg % 2]
                        for jb in range(4 * gg, 4 * gg + 4):
                            q0 = (jb - 4 * gg) * P
                            at = atps.tile([P, D + 1], F32, name="at_ps",
                                           tag="at_ps")
                            for tb in range(jb + 1):
                                nc.tensor.matmul(
                                    at[:], ex[:, tb, q0:q0 + P],
                                    v_sb[:, tb % 2, tb // 2, :D + 1],
                                    start=(tb == 0), stop=(tb == jb))
                            recip = asmall.tile([P, 1], F32, name="recip",
                                                tag="recip")
                            nc.vector.reciprocal(recip[:], at[:, D:D + 1])
                            a_sb = asmall.tile([P, P], BF, name="a_sb",
                                               tag="a_sb")
                            nc.vector.tensor_scalar_mul(a_sb[:], at[:, :D],
                                                        recip[:])
                            tp = trps.tile([P, P], BF, name="tp_at", tag="tp_at")
                            nc.tensor.transpose(tp[:], a_sb[:], ident[:])
                            nc.vector.tensor_copy(
                                attnT[:, m, jb * P:(jb + 1) * P], tp[:])

        # ---------- partial out-projection + 2 ReduceScatters ----------
        wpa_sb = res.tile([P, 2, 4, 512], BF, name="wpa_sb")
        with tc.tile_wait_until(0.110):
            nc.gpsimd.dma_start(wpa_sb[:], wpa[:])
        with tc.tile_pool(name="pout", bufs=6) as pout_pool, \
             tc.tile_pool(name="pps", bufs=5, space="PSUM") as pps_pool:
            for phase in range(2):  # even row-blocks, then odd
                for jb in range(phase, NBLK, 2):
                    for n in range(4):
                        ps = pps_pool.tile([P, 512], F32, name="p_ps",
                                           tag="p_ps")
                        for mm in range(2):
                            nc.tensor.matmul(
                                ps[:], attnT[:, mm, jb * P:(jb + 1) * P],
                                wpa_sb[:, mm, n, :],
                                start=(mm == 0), stop=(mm == 1))
                        po = pout_pool.tile([P, 512], BF, name="po", tag="po")
                        if (jb * 4 + n) % 2 == 0:
                            nc.vector.tensor_copy(po[:], ps[:])
                        else:
                            nc.scalar.copy(po[:], ps[:])
                        nc.sync.dma_start(
                            rs_in[phase][jb // 2, :, n * 512:(n + 1) * 512],
                            po[:])
                nc.gpsimd.collective_compute(
                    "ReduceScatter", ALU.add,
                    replica_groups=[list(range(NCORES))],
                    ins=[rs_in[phase].opt()], outs=[rs_out[phase].opt()])

        # ---------- residual + LN2 + FFN (both 128-row chunks per tile) ----
        with tc.tile_pool(name="ffn_big", bufs=1) as fbig:
            gT = [fbig.tile([P, FC2, P], BF, name=f"gT_sb{j}") for j in range(2)]
            h2T = [fbig.tile([P, EC, P], BF, name=f"h2T_sb{j}") for j in range(2)]
            _ffn(nc, tc, res, small, x_sb, rs_out, out_own, wfc, wpf, ident,
                 eps_tile, gT, h2T)


def _ffn(nc, tc, res, small, x_sb, rs_out, out_own, wfc, wpf, ident, eps_tile,
         gT, h2T):
        with tc.tile_pool(name="ffn", bufs=2) as fpool, \
             tc.tile_pool(name="tps2", bufs=3, space="PSUM") as tps2:
            for j in range(2):
                rs_sb = fpool.tile([P, E], BF, name="rs_sb", tag="rs_sb")
                nc.sync.dma_start(rs_sb[:], rs_out[j][:])
                nc.vector.tensor_add(out=x_sb[j][:], in0=x_sb[j][:],
                                     in1=rs_sb[:])
                h2_bf = fpool.tile([P, E], BF, name="h2_bf", tag="h2_bf")
                _layer_norm(nc, small, x_sb[j], h2_bf, eps_tile, f"ln2_{j}")
                for e in range(EC):
                    tp = tps2.tile([P, P], BF, name="tp2", tag="tp2")
                    nc.tensor.transpose(tp[:], h2_bf[:, e * P:(e + 1) * P],
                                        ident[:])
                    nc.vector.tensor_copy(h2T[j][:, e, :], tp[:])
        with tc.tile_pool(name="wfc_sb", bufs=12) as wfc_pool, \
             tc.tile_pool(name="fcps", bufs=4, space="PSUM") as fcps:
            for f2 in range(FC2):
                wt = wfc_pool.tile([P, EC, P], BF, name="wfc_t", tag="wfc_t")
                if f2 < 12:
                    with tc.tile_wait_until(0.150):
                        nc.scalar.dma_start(wt[:], wfc[f2])
                else:
                    nc.scalar.dma_start(wt[:], wfc[f2])
                for j in range(2):
                    ps = fcps.tile([P, P], F32, name="fc_ps", tag="fc_ps")
                    for e in range(EC):
                        nc.tensor.matmul(ps[:], wt[:, e, :], h2T[j][:, e, :],
                                         start=(e == 0), stop=(e == EC - 1))
                    nc.scalar.activation(out=gT[j][:, f2, :], in_=ps[:],
                                         func=AF.Gelu_apprx_tanh)
        with tc.tile_pool(name="wpf_sb", bufs=6) as wpf_pool, \
             tc.tile_pool(name="pfps", bufs=1, space="PSUM") as pfps:
            ps2 = [[pfps.tile([P, 512], F32, name=f"pf_ps{j}_{n}",
                              tag=f"pf_ps{j}_{n}") for n in range(4)]
                   for j in range(2)]
            for f2 in range(FC2):
                wt = wpf_pool.tile([P, 4, 512], BF, name="wpf_t", tag="wpf_t")
                nc.sync.dma_start(wt[:], wpf[f2])
                for j in range(2):
                    for n in range(4):
                        nc.tensor.matmul(ps2[j][n][:], gT[j][:, f2, :],
                                         wt[:, n, :],
                                         start=(f2 == 0), stop=(f2 == FC2 - 1))
            for j in range(2):
                for n in range(4):
                    nc.vector.tensor_add(
                        out=x_sb[j][:, n * 512:(n + 1) * 512],
                        in0=x_sb[j][:, n * 512:(n + 1) * 512],
                        in1=ps2[j][n][:])
                    nc.sync.dma_start(out_own[j][:, n * 512:(n + 1) * 512],
                                      x_sb[j][:, n * 512:(n + 1) * 512])


# ------------------------------------------------------------------
# host side
# ------------------------------------------------------------------
_BF = ml_dtypes.bfloat16


def _prep_shared(ln1_w, ln2_w, w_attn, w_fc, w_proj_ffn):
    w_attn = (w_attn * ln1_w[None, :]).astype(np.float32)
    w_fc = (w_fc * ln2_w[None, :]).astype(np.float32)
    wq = w_attn[:E]
    wk = w_attn[E:2 * E]
    wv_rows = w_attn[2 * E:]

    # wqk[m, i, e, ec, d] = src[head][d, 128ec+e]
    wqk = np.empty((2, 16, P, EC, P), dtype=_BF)
    for mm in range(2):
        for i in range(16):
            h = 8 * mm + (i % 8)
            src = wk if i < 8 else wq
            blk = src[h * P:(h + 1) * P]          # [d, E]
            wqk[mm, i] = blk.reshape(P, EC, P).transpose(2, 1, 0).astype(_BF)
    # wv[m, ec, e, cc, f] = wv_rows[1024m + 256cc + f, 128ec + e]
    wv = np.ascontiguousarray(
        wv_rows.reshape(2, 4, 256, EC, P).transpose(0, 3, 4, 1, 2)).astype(_BF)
    wfc_t = np.ascontiguousarray(
        w_fc.reshape(FC2, P, EC, P).transpose(0, 3, 2, 1)).astype(_BF)
    wpf = np.ascontiguousarray(
        w_proj_ffn.reshape(4, 512, FC2, P).transpose(2, 3, 0, 1)).astype(_BF)

    t = np.arange(P)[:, None, None]
    dtb = np.arange(4)[None, :, None]
    qf = np.arange(512)[None, None, :]
    mask = (dtb * P + t <= qf).astype(np.float32).astype(_BF)
    ident = np.eye(P, dtype=np.float32).astype(_BF)
    return wqk, wv, wfc_t, wpf, np.ascontiguousarray(mask), ident


def _core_wpa(w_proj_attn, c):
    # wpa[i, mm, n, f] = w_proj_attn[512n+f, 128*(8mm+c)+i]
    cols = np.stack([w_proj_attn[:, (8 * mm + c) * P:(8 * mm + c + 1) * P]
                     for mm in range(2)], axis=0)      # [2, E, P]
    return np.ascontiguousarray(
        cols.reshape(2, 4, 512, P).transpose(3, 0, 1, 2)).astype(_BF)


_CACHE = {}


def _get_program():
    if "nc" not in _CACHE:
        _CACHE["nc"] = build_program()
    return _CACHE["nc"]


def make_in_maps(x, ln1_w, ln2_w, w_attn, w_proj_attn, w_fc, w_proj_ffn):
    wqk, wv, wfc_t, wpf, mask, ident = _prep_shared(
        np.asarray(ln1_w, np.float32), np.asarray(ln2_w, np.float32),
        np.asarray(w_attn, np.float32), np.asarray(w_fc, np.float32),
        np.asarray(w_proj_ffn, np.float32))
    wpa_full = np.asarray(w_proj_attn, np.float32)
    xb = np.ascontiguousarray(np.asarray(x, np.float32).reshape(NBLK, P, E))
    in_maps = []
    for c in range(NCORES):
        in_maps.append({
            "x_own": np.ascontiguousarray(xb[2 * c:2 * c + 2]),
            "wqk": wqk, "wv": wv, "wpa": _core_wpa(wpa_full, c),
            "wfc": wfc_t, "wpf": wpf, "mask": mask, "ident": ident,
        })
    return in_maps


def kernel(x, ln1_w, ln2_w, w_attn, w_proj_attn, w_fc, w_proj_ffn):
    nc = _get_program()
    in_maps = make_in_maps(x, ln1_w, ln2_w, w_attn, w_proj_attn, w_fc,
                           w_proj_ffn)
    res = run_bass_kernel_spmd(nc, in_maps, core_ids=list(range(NCORES)))
    out = np.empty((S, E), np.float32)
    for c in range(NCORES):
        blk = res.results[c]["out_own"]
        out[2 * c * P:(2 * c + 1) * P] = blk[0]
        out[(2 * c + 1) * P:(2 * c + 2) * P] = blk[1]
    return out


if __name__ == "__main__":
    rng = np.random.default_rng(0)
    ins = {
        "x": rng.standard_normal((S, E), dtype=np.float32),
        "ln1_w": np.ones(E, np.float32),
        "ln2_w": np.ones(E, np.float32),
        "w_attn": (rng.standard_normal((3 * E, E), dtype=np.float32) * 0.02),
        "w_proj_attn": (rng.standard_normal((E, E), dtype=np.float32) * 0.02),
        "w_fc": (rng.standard_normal((FH, E), dtype=np.float32) * 0.02),
        "w_proj_ffn": (rng.standard_normal((E, FH), dtype=np.float32) * 0.02),
    }
    out = kernel(**ins)
    print("ran:", out.shape, out.dtype, np.abs(out).max())


# revision 35
# speedup vs baseline: 1.0386x; 1.0386x over previous
"""Trainium2 Bass kernel for a GPT-style decoder block (S=2048, E=2048, H=16, D=128).

Strategy (8 NeuronCores, SPMD-uniform program):
- Sequence-parallel LN1/QKV: core c owns rows 256c..256c+255 and computes
  q,k,v (bf16) for all 16 heads of its rows.
- Two AllToAlls re-shard (q,k,v) from sequence-sharded to head-sharded:
  round m moves heads 8m..8m+7 (head 8m+c lands on core c).
- Head-sharded causal attention: each core runs full-sequence attention
  for its 2 heads, computing only the causal lower-triangle score blocks
  (identical work on every core, so the program stays uniform).
- Attention out-projection is computed as per-core partials (contraction
  over the 2 owned heads only) and summed with two pipelined bf16
  ReduceScatters (even row-blocks first, then odd), which also return
  the result to sequence sharding.
- LN2 + FFN run sequence-parallel; both 128-row chunks are processed per
  streamed weight tile so wfc/wpf are read from HBM exactly once.

Matmuls accumulate in fp32 PSUM; the residual stream stays fp32 in SBUF.
LayerNorm scales fold into the following weights host-side.
"""

import numpy as np
import ml_dtypes

import concourse.mybir as mybir
import concourse.tile as tile
from concourse import bacc
from concourse.bass_utils import run_bass_kernel_spmd

P = 128
S, E, H, D = 2048, 2048, 16, 128
FH = 4 * E
NCORES = 8
NBLK = 16            # S / P row blocks
EC = 16              # E / P contraction chunks
FC2 = 64             # FH / P
BF = mybir.dt.bfloat16
F32 = mybir.dt.float32
EPS = 1e-5
SCALE = 1.0 / np.sqrt(D)
AF = mybir.ActivationFunctionType
ALU = mybir.AluOpType

KQB = P * 256        # bf16 elements of one [128, 256] k/q block in a chunk
CH = 3 * KQB         # chunk elements: k + q + v (all bf16)


def _layer_norm(nc, small, x_sb, h_out, eps_tile, tag):
    """h_out[P, E] = (x - mean(x)) * rsqrt(var(x) + eps), rowwise over E."""
    stats = small.tile([P, 4, 6], F32, name=f"stats_{tag}", tag="stats")
    for g in range(4):
        nc.vector.bn_stats(out=stats[:, g, :], in_=x_sb[:, g * 512:(g + 1) * 512])
    mv = small.tile([P, 2], F32, name=f"mv_{tag}", tag="mv")
    nc.vector.bn_aggr(out=mv[:], in_=stats[:])
    std = small.tile([P, 1], F32, name=f"std_{tag}", tag="std")
    nc.scalar.activation(out=std[:], in_=mv[:, 1:2], func=AF.Sqrt,
                         bias=eps_tile[:], scale=1.0)
    rstd = small.tile([P, 1], F32, name=f"rstd_{tag}", tag="rstd")
    nc.vector.reciprocal(out=rstd[:], in_=std[:])
    nc.vector.tensor_scalar(
        out=h_out[:], in0=x_sb[:], scalar1=mv[:, 0:1], scalar2=rstd[:],
        op0=ALU.subtract, op1=ALU.mult)


def build_program():
    nc = bacc.Bacc()

    x_own = nc.dram_tensor("x_own", [2, P, E], F32, kind="ExternalInput")
    # wqk[m][i][e, ec, d]: i<8 -> k head 8m+i, i>=8 -> q head 8m+i-8
    wqk = nc.dram_tensor("wqk", [2, 16, P, EC, P], BF, kind="ExternalInput")
    # wv[m][ec][e, cc, f]: round-m v cols 1024m+256cc+f
    wv = nc.dram_tensor("wv", [2, EC, P, 4, 256], BF, kind="ExternalInput")
    # wpa[i, mm, n, f] = w_proj_attn[512n+f, 128*(8mm+c)+i]  (per-core)
    wpa = nc.dram_tensor("wpa", [P, 2, 4, 512], BF, kind="ExternalInput")
    # wfc[f2][e, ec, f] = (w_fc*ln2)[128f2+f, 128ec+e]
    wfc = nc.dram_tensor("wfc", [FC2, P, EC, P], BF, kind="ExternalInput")
    # wpf[f2][i, n, f] = w_proj_ffn[512n+f, 128f2+i]
    wpf = nc.dram_tensor("wpf", [FC2, P, 4, 512], BF, kind="ExternalInput")
    # mask[t, dtb, qf] = 1 if 128*dtb + t <= qf else 0 (rank-independent)
    mask_in = nc.dram_tensor("mask", [P, 4, 512], BF, kind="ExternalInput")
    ident_in = nc.dram_tensor("ident", [P, P], BF, kind="ExternalInput")
    out_own = nc.dram_tensor("out_own", [2, P, E], F32, kind="ExternalOutput")

    with tile.TileContext(nc) as tc:
        _body(nc, tc, x_own, wqk, wv, wpa, wfc, wpf, mask_in, ident_in, out_own)
    nc.finalize()
    return nc


def _body(nc, tc, x_own, wqk, wv, wpa, wfc, wpf, mask_in, ident_in, out_own):
    with tc.tile_pool(name="resident", bufs=1) as res, \
         tc.tile_pool(name="small", bufs=4) as small, \
         tc.tile_pool(name="dram", bufs=1, space="DRAM") as dram:

        eps_tile = small.tile([P, 1], F32, name="eps_tile", tag="eps")
        nc.vector.memset(eps_tile[:], EPS)
        x_sb = []
        for j in range(2):
            x = res.tile([P, E], F32, name=f"x_sb{j}")
            nc.sync.dma_start(x[:], x_own[j])
            x_sb.append(x)
        ident = res.tile([P, P], BF, name="ident_sb")
        nc.sync.dma_start(ident[:], ident_in[:])
        mask_sb = res.tile([P, 4, 512], BF, name="mask_sb")

        hT = res.tile([P, EC, 256], BF, name="hT_sb")
        attnT = res.tile([P, 2, S], BF, name="attnT_sb")

        a2a_in = [dram.tile([NCORES, CH], BF, name=f"a2a_in{m}")
                  for m in range(2)]
        a2a_out = [dram.tile([NCORES, CH], BF, name=f"a2a_out{m}")
                   for m in range(2)]
        rs_in = [dram.tile([NCORES, P, E], BF, name=f"rs_in{i}") for i in range(2)]
        rs_out = [dram.tile([P, E], BF, name=f"rs_out{i}") for i in range(2)]

        # ---------- LN1 -> hT (bf16) ----------
        with tc.tile_pool(name="hbf", bufs=2) as hbf_pool, \
             tc.tile_pool(name="tps1", bufs=4, space="PSUM") as tps1:
            h_bfs = []
            for j in range(2):
                h_bf = hbf_pool.tile([P, E], BF, name="h_bf", tag="h_bf")
                _layer_norm(nc, small, x_sb[j], h_bf, eps_tile, f"ln1_{j}")
                h_bfs.append(h_bf)
            for e in range(EC):
                for j in range(2):
                    tp = tps1.tile([P, P], BF, name="tp1", tag="tp1")
                    nc.tensor.transpose(tp[:], h_bfs[j][:, e * P:(e + 1) * P],
                                        ident[:])
                    nc.vector.tensor_copy(hT[:, e, j * P:(j + 1) * P], tp[:])

        with tc.tile_wait_until(0.095):
            nc.scalar.dma_start(mask_sb[:], mask_in[:])

        # ---------- QKV rounds + AllToAll ----------
        # Per-round weights land in SBUF via 3 big ordered DMAs on the sync
        # queue (the in-order SEQ serializes them ahead of payload writes),
        # so the PE runs unthrottled once data arrives.
        with tc.tile_pool(name="wv_sb", bufs=2) as wv_pool, \
             tc.tile_pool(name="wqk_sb", bufs=2) as wqk_pool:
            wv_t = []
            wqk_t = []
            for m in range(2):
                from contextlib import nullcontext
                gate = tc.tile_wait_until(0.055) if m == 1 else nullcontext()
                with gate:
                    wvt = wv_pool.tile([P, EC, 4, 256], BF, name=f"wv_t{m}",
                                       tag="wv_t")
                    nc.sync.dma_start(wvt[:],
                                      wv[m].rearrange("c e f g -> e c f g"))
                    halves = []
                    for hh in range(2):
                        wt = wqk_pool.tile([P, 8, EC, P], BF,
                                           name=f"wqk_t{m}{hh}", tag="wqk_t")
                        nc.sync.dma_start(
                            wt[:], wqk[m, 8 * hh:8 * hh + 8].rearrange(
                                "c e f g -> e c f g"))
                        halves.append(wt)
                wv_t.append(wvt)
                wqk_t.append(halves)

                with tc.tile_pool(name=f"kvtmp{m}", bufs=4) as kv_pool, \
                     tc.tile_pool(name=f"qkps{m}", bufs=3, space="PSUM") as qkps, \
                     tc.tile_pool(name=f"vps{m}", bufs=1, space="PSUM") as vps:
                    ps_v = [[vps.tile([P, 512], F32, name=f"v_ps{j}_{ccp}",
                                      tag=f"v_ps{j}_{ccp}") for ccp in range(2)]
                            for j in range(2)]
                    for e in range(EC):
                        for j in range(2):
                            for ccp in range(2):
                                nc.tensor.matmul(
                                    ps_v[j][ccp][:],
                                    hT[:, e, j * P:(j + 1) * P],
                                    wvt[:, e, 2 * ccp:2 * ccp + 2, :],
                                    start=(e == 0), stop=(e == EC - 1))
                    for j in range(2):
                        for ccp in range(2):
                            v_sb = kv_pool.tile([P, 512], BF, name="v_sb",
                                                tag="v_sb")
                            nc.vector.tensor_copy(v_sb[:], ps_v[j][ccp][:])
                            for half in range(4):
                                r = 4 * ccp + half
                                dst = a2a_in[m][r, 2 * KQB + j * (KQB // 2):
                                                2 * KQB + (j + 1) * (KQB // 2)]
                                nc.sync.dma_start(
                                    dst.rearrange("(t d) -> t d", t=P),
                                    v_sb[:, half * P:(half + 1) * P])
                    for i in range(16):
                        wt = halves[i // 8]
                        ps = qkps.tile([P, 256], F32, name="qk_ps", tag="qk_ps")
                        for e in range(EC):
                            nc.tensor.matmul(ps[:], wt[:, i % 8, e, :],
                                             hT[:, e, :],
                                             start=(e == 0), stop=(e == EC - 1))
                        kq_sb = kv_pool.tile([P, 256], BF, name="kq_sb",
                                             tag="kq_sb")
                        nc.scalar.copy(kq_sb[:], ps[:])
                        r, slot = i % 8, i // 8
                        dst = a2a_in[m][r, slot * KQB:(slot + 1) * KQB]
                        nc.sync.dma_start(
                            dst.rearrange("(d t) -> d t", d=P), kq_sb[:])
            nc.gpsimd.collective_compute(
                    "AllToAll", ALU.bypass,
                    replica_groups=[list(range(NCORES))],
                    ins=[a2a_in[m].opt()], outs=[a2a_out[m].opt()])

        # ---------- attention: 2 heads, causal-only blocks ----------
        with tc.tile_pool(name="kqv_in", bufs=2) as kqv_pool, \
             tc.tile_pool(name="expt", bufs=2) as exp_pool, \
             tc.tile_pool(name="attn_small", bufs=4) as asmall, \
             tc.tile_pool(name="scps", bufs=3, space="PSUM") as scps, \
             tc.tile_pool(name="atps", bufs=2, space="PSUM") as atps, \
             tc.tile_pool(name="trps", bufs=2, space="PSUM") as trps:
            for m in range(2):
                kT = kqv_pool.tile([P, NBLK, P], BF, name="kT", tag="kT")
                nc.sync.dma_start(
                    kT[:],
                    a2a_out[m][:, :KQB].rearrange("r (d t) -> d r t", d=P))
                qT = kqv_pool.tile([P, S], BF, name="qT", tag="qT")
                nc.sync.dma_start(
                    qT[:],
                    a2a_out[m][:, KQB:2 * KQB].rearrange(
                        "r (d t) -> d r t", d=P))
                v_sb = kqv_pool.tile([P, 2, NCORES, D + 2], BF,
                                     name="v_all", tag="v_all")
                nc.vector.memset(v_sb[:, :, :, D:D + 1], 1.0)
                for j in range(2):
                    nc.scalar.dma_start(
                        v_sb[:, j, :, :D],
                        a2a_out[m][:, 2 * KQB + j * (KQB // 2):
                                   2 * KQB + (j + 1) * (KQB // 2)].rearrange(
                            "r (t d) -> t r d", t=P))

                expT = [None, None]  # pipeline: scores(g) || attnV(g-1)
                for g in range(5):
                    if g < 4:
                        ex = exp_pool.tile([P, NBLK, 512], BF, name="expT",
                                           tag="expT")
                        expT[g % 2] = ex
                        for tb in range(4 * g + 4):
                            dtb = tb - 4 * g
                            q0 = max(dtb, 0) * P  # causal: skip left of diag
                            ps = scps.tile([P, 512], F32, name="sc_ps",
                                           tag="sc_ps")
                            nc.tensor.matmul(
                                ps[:, q0:], kT[:, tb, :],
                                qT[:, g * 512 + q0:(g + 1) * 512],
                                start=True, stop=True)
                            nc.scalar.activation(
                                out=ex[:, tb, q0:], in_=ps[:, q0:],
                                func=AF.Exp, scale=float(SCALE))
                            if dtb >= 0:
                                nc.vector.tensor_mul(
                                    ex[:, tb, q0:], ex[:, tb, q0:],
                                    mask_sb[:, dtb, q0:])
                    if g > 0:
                        gg = g - 1
                        ex = expT[gg % 2]
                        for jb in range(4 * gg, 4 * gg + 4):
                            q0 = (jb - 4 * gg) * P
                            at = atps.tile([P, D + 1], F32, name="at_ps",
                                           tag="at_ps")
                            for tb in range(jb + 1):
                                nc.tensor.matmul(
                                    at[:], ex[:, tb, q0:q0 + P],
                                    v_sb[:, tb % 2, tb // 2, :D + 1],
                                    start=(tb == 0), stop=(tb == jb))
                            recip = asmall.tile([P, 1], F32, name="recip",
                                                tag="recip")
                            nc.vector.reciprocal(recip[:], at[:, D:D + 1])
                            a_sb = asmall.tile([P, P], BF, name="a_sb",
                                               tag="a_sb")
                            nc.vector.tensor_scalar_mul(a_sb[:], at[:, :D],
                                                        recip[:])
                            tp = trps.tile([P, P], BF, name="tp_at", tag="tp_at")
                            nc.tensor.transpose(tp[:], a_sb[:], ident[:])
                            nc.vector.tensor_copy(
                                attnT[:, m, jb * P:(jb + 1) * P], tp[:])

        # ---------- partial out-projection + 2 ReduceScatters ----------
        wpa_sb = res.tile([P, 2, 4, 512], BF, name="wpa_sb")
        with tc.tile_wait_until(0.110):
            nc.gpsimd.dma_start(wpa_sb[:], wpa[:])
        with tc.tile_pool(name="pout", bufs=6) as pout_pool, \
             tc.tile_pool(name="pps", bufs=5, space="PSUM") as pps_pool:
            for phase in range(2):  # even row-blocks, then odd
                for jb in range(phase, NBLK, 2):
                    for n in range(4):
                        ps = pps_pool.tile([P, 512], F32, name="p_ps",
                                           tag="p_ps")
                        for mm in range(2):
                            nc.tensor.matmul(
                                ps[:], attnT[:, mm, jb * P:(jb + 1) * P],
                                wpa_sb[:, mm, n, :],
                                start=(mm == 0), stop=(mm == 1))
                        po = pout_pool.tile([P, 512], BF, name="po", tag="po")
                        if (jb * 4 + n) % 2 == 0:
                            nc.vector.tensor_copy(po[:], ps[:])
                        else:
                            nc.scalar.copy(po[:], ps[:])
                        nc.sync.dma_start(
                            rs_in[phase][jb // 2, :, n * 512:(n + 1) * 512],
                            po[:])
                nc.gpsimd.collective_compute(
                    "ReduceScatter", ALU.add,
                    replica_groups=[list(range(NCORES))],
                    ins=[rs_in[phase].opt()], outs=[rs_out[phase].opt()])

        # ---------- residual + LN2 + FFN (both 128-row chunks per tile) ----
        with tc.tile_pool(name="ffn_big", bufs=1) as fbig:
            gT = [fbig.tile([P, FC2, P], BF, name=f"gT_sb{j}") for j in range(2)]
            h2T = [fbig.tile([P, EC, P], BF, name=f"h2T_sb{j}") for j in range(2)]
            _ffn(nc, tc, res, small, x_sb, rs_out, out_own, wfc, wpf, ident,
                 eps_tile, gT, h2T)


def _ffn(nc, tc, res, small, x_sb, rs_out, out_own, wfc, wpf, ident, eps_tile,
         gT, h2T):
        EARLY = 20  # fc tiles run for chunk 0 alone while RS1 is in flight
        with tc.tile_pool(name="ffn", bufs=2) as fpool, \
             tc.tile_pool(name="tps2", bufs=3, space="PSUM") as tps2, \
             tc.tile_pool(name="wfc_sb", bufs=12) as wfc_pool, \
             tc.tile_pool(name="fcps", bufs=4, space="PSUM") as fcps:
            def resid_ln2(j):
                rs_sb = fpool.tile([P, E], BF, name="rs_sb", tag="rs_sb")
                nc.sync.dma_start(rs_sb[:], rs_out[j][:])
                nc.vector.tensor_add(out=x_sb[j][:], in0=x_sb[j][:],
                                     in1=rs_sb[:])
                h2_bf = fpool.tile([P, E], BF, name="h2_bf", tag="h2_bf")
                _layer_norm(nc, small, x_sb[j], h2_bf, eps_tile, f"ln2_{j}")
                for e in range(EC):
                    tp = tps2.tile([P, P], BF, name="tp2", tag="tp2")
                    nc.tensor.transpose(tp[:], h2_bf[:, e * P:(e + 1) * P],
                                        ident[:])
                    nc.vector.tensor_copy(h2T[j][:, e, :], tp[:])
            def fc_one(wt, f2, j):
                ps = fcps.tile([P, P], F32, name="fc_ps", tag="fc_ps")
                for e in range(EC):
                    nc.tensor.matmul(ps[:], wt[:, e, :], h2T[j][:, e, :],
                                     start=(e == 0), stop=(e == EC - 1))
                nc.scalar.activation(out=gT[j][:, f2, :], in_=ps[:],
                                     func=AF.Gelu_apprx_tanh)
            resid_ln2(0)
            for f2 in range(EARLY):
                wt = wfc_pool.tile([P, EC, P], BF, name="wfc_t", tag="wfc_t")
                with tc.tile_wait_until(0.150):
                    nc.scalar.dma_start(wt[:], wfc[f2])
                fc_one(wt, f2, 0)
            resid_ln2(1)
            for f2 in range(FC2):
                wt = wfc_pool.tile([P, EC, P], BF, name="wfc_t", tag="wfc_t")
                nc.scalar.dma_start(wt[:], wfc[f2])
                fc_one(wt, f2, 1)
                if f2 >= EARLY:
                    fc_one(wt, f2, 0)
        with tc.tile_pool(name="wpf_sb", bufs=6) as wpf_pool, \
             tc.tile_pool(name="pfps", bufs=1, space="PSUM") as pfps:
            ps2 = [[pfps.tile([P, 512], F32, name=f"pf_ps{j}_{n}",
                              tag=f"pf_ps{j}_{n}") for n in range(4)]
                   for j in range(2)]
            for f2 in range(FC2):
                wt = wpf_pool.tile([P, 4, 512], BF, name="wpf_t", tag="wpf_t")
                nc.sync.dma_start(wt[:], wpf[f2])
                for j in range(2):
                    for n in range(4):
                        nc.tensor.matmul(ps2[j][n][:], gT[j][:, f2, :],
                                         wt[:, n, :],
                                         start=(f2 == 0), stop=(f2 == FC2 - 1))
            for j in range(2):
                for n in range(4):
                    nc.vector.tensor_add(
                        out=x_sb[j][:, n * 512:(n + 1) * 512],
                        in0=x_sb[j][:, n * 512:(n + 1) * 512],
                        in1=ps2[j][n][:])
                    nc.sync.dma_start(out_own[j][:, n * 512:(n + 1) * 512],
                                      x_sb[j][:, n * 512:(n + 1) * 512])


# ------------------------------------------------------------------
# host side
# ------------------------------------------------------------------
_BF = ml_dtypes.bfloat16


def _prep_shared(ln1_w, ln2_w, w_attn, w_fc, w_proj_ffn):
    w_attn = (w_attn * ln1_w[None, :]).astype(np.float32)
    w_fc = (w_fc * ln2_w[None, :]).astype(np.float32)
    wq = w_attn[:E]
    wk = w_attn[E:2 * E]
    wv_rows = w_attn[2 * E:]

    # wqk[m, i, e, ec, d] = src[head][d, 128ec+e]
    wqk = np.empty((2, 16, P, EC, P), dtype=_BF)
    for mm in range(2):
        for i in range(16):
            h = 8 * mm + (i % 8)
            src = wk if i < 8 else wq
            blk = src[h * P:(h + 1) * P]          # [d, E]
            wqk[mm, i] = blk.reshape(P, EC, P).transpose(2, 1, 0).astype(_BF)
    # wv[m, ec, e, cc, f] = wv_rows[1024m + 256cc + f, 128ec + e]
    wv = np.ascontiguousarray(
        wv_rows.reshape(2, 4, 256, EC, P).transpose(0, 3, 4, 1, 2)).astype(_BF)
    wfc_t = np.ascontiguousarray(
        w_fc.reshape(FC2, P, EC, P).transpose(0, 3, 2, 1)).astype(_BF)
    wpf = np.ascontiguousarray(
        w_proj_ffn.reshape(4, 512, FC2, P).transpose(2, 3, 0, 1)).astype(_BF)

    t = np.arange(P)[:, None, None]
    dtb = np.arange(4)[None, :, None]
    qf = np.arange(512)[None, None, :]
    mask = (dtb * P + t <= qf).astype(np.float32).astype(_BF)
    ident = np.eye(P, dtype=np.float32).astype(_BF)
    return wqk, wv, wfc_t, wpf, np.ascontiguousarray(mask), ident


def _core_wpa(w_proj_attn, c):
    # wpa[i, mm, n, f] = w_proj_attn[512n+f, 128*(8mm+c)+i]
    cols = np.stack([w_proj_attn[:, (8 * mm + c) * P:(8 * mm + c + 1) * P]
                     for mm in range(2)], axis=0)      # [2, E, P]
    return np.ascontiguousarray(
        cols.reshape(2, 4, 512, P).transpose(3, 0, 1, 2)).astype(_BF)


_CACHE = {}


def _get_program():
    if "nc" not in _CACHE:
        _CACHE["nc"] = build_program()
    return _CACHE["nc"]


def make_in_maps(x, ln1_w, ln2_w, w_attn, w_proj_attn, w_fc, w_proj_ffn):
    wqk, wv, wfc_t, wpf, mask, ident = _prep_shared(
        np.asarray(ln1_w, np.float32), np.asarray(ln2_w, np.float32),
        np.asarray(w_attn, np.float32), np.asarray(w_fc, np.float32),
        np.asarray(w_proj_ffn, np.float32))
    wpa_full = np.asarray(w_proj_attn, np.float32)
    xb = np.ascontiguousarray(np.asarray(x, np.float32).reshape(NBLK, P, E))
    in_maps = []
    for c in range(NCORES):
        in_maps.append({
            "x_own": np.ascontiguousarray(xb[2 * c:2 * c + 2]),
            "wqk": wqk, "wv": wv, "wpa": _core_wpa(wpa_full, c),
            "wfc": wfc_t, "wpf": wpf, "mask": mask, "ident": ident,
        })
    return in_maps


def kernel(x, ln1_w, ln2_w, w_attn, w_proj_attn, w_fc, w_proj_ffn):
    nc = _get_program()
    in_maps = make_in_maps(x, ln1_w, ln2_w, w_attn, w_proj_attn, w_fc,
                           w_proj_ffn)
    res = run_bass_kernel_spmd(nc, in_maps, core_ids=list(range(NCORES)))
    out = np.empty((S, E), np.float32)
    for c in range(NCORES):
        blk = res.results[c]["out_own"]
        out[2 * c * P:(2 * c + 1) * P] = blk[0]
        out[(2 * c + 1) * P:(2 * c + 2) * P] = blk[1]
    return out


if __name__ == "__main__":
    rng = np.random.default_rng(0)
    ins = {
        "x": rng.standard_normal((S, E), dtype=np.float32),
        "ln1_w": np.ones(E, np.float32),
        "ln2_w": np.ones(E, np.float32),
        "w_attn": (rng.standard_normal((3 * E, E), dtype=np.float32) * 0.02),
        "w_proj_attn": (rng.standard_normal((E, E), dtype=np.float32) * 0.02),
        "w_fc": (rng.standard_normal((FH, E), dtype=np.float32) * 0.02),
        "w_proj_ffn": (rng.standard_normal((E, FH), dtype=np.float32) * 0.02),
    }
    out = kernel(**ins)
    print("ran:", out.shape, out.dtype, np.abs(out).max())
